# revision 3
# baseline (speedup 1.0000x reference)
"""Causal attention (LN -> QKV -> 16-head causal attn -> out-proj) on 8 TRN2 cores.

Sharding: core c = (batch b=c//4, head-group g=c%4). Each core runs its batch's
LayerNorm + a 4-head slice of QKV / attention / out-projection. The out-proj
partials (column-split over the inner dim) are summed on the host per batch.

All device I/O is bf16 (host pre-casts weights/x, host accumulates bf16 out
partials in fp32) to halve HBM traffic.

Device layout (per core):
  xnT  4x [128, 8, 512] bf16  normalized input, transposed (DIM on partitions)
  QT/KT pair tiles [128, 2048] bf16  (two heads stacked: head-dim on partitions)
  V    [128, 16, 4, 65] bf16  keys-on-partitions; 65th col = 1.0 so PV's lhsT
                              also accumulates softmax denominators
  Phase B runs per (q-chunk of 512, head-pair): S^T psum [128, 2, 512] (both
  heads packed, double-buffered), one exp instruction per key-block covering
  both heads straight from PSUM, causal staircase at 128 granularity, diagonal
  mask on GpSimd, PV software-pipelined one key-block behind S so the PE never
  waits on the activation engine. Out-projection for chunk c is emitted inside
  chunk c+1's attention to keep the PE dense and spread the output DMA.
"""

import numpy as np
import ml_dtypes

import concourse.bass as bass
import concourse.mybir as mybir
import concourse.tile as tile
from concourse import bacc
from concourse.bass_utils import run_bass_kernel_spmd
from concourse.masks import make_identity

B, N, DIM, HEADS, DIM_HEAD = 2, 2048, 1024, 16, 64
INNER = HEADS * DIM_HEAD
H_LOC = 4                      # heads per core
N_CORES = 8
P = 128
NB = N // P                    # 16 seq blocks
KB = DIM // P                  # 8 dim blocks
CH = 512                       # psum-bank-sized q chunk
NCH = N // CH                  # 4 q chunks
SCALE = DIM_HEAD ** -0.5
LN_EPS = 1e-5
BFNP = ml_dtypes.bfloat16

F32 = mybir.dt.float32
BF16 = mybir.dt.bfloat16
AF = mybir.ActivationFunctionType
ALU = mybir.AluOpType


def build_nc():
    from contextlib import ExitStack

    nc = bacc.Bacc(None, target_bir_lowering=False, debug=False)

    x_d = nc.dram_tensor("x", [N, DIM], BF16, kind="ExternalInput")
    wq_d = nc.dram_tensor("wq", [DIM, H_LOC * DIM_HEAD], BF16, kind="ExternalInput")
    wk_d = nc.dram_tensor("wk", [DIM, H_LOC * DIM_HEAD], BF16, kind="ExternalInput")
    wv_d = nc.dram_tensor("wv", [DIM, H_LOC * DIM_HEAD], BF16, kind="ExternalInput")
    wo_d = nc.dram_tensor("wo", [H_LOC * DIM_HEAD, DIM], BF16, kind="ExternalInput")
    bq_d = nc.dram_tensor("bq", [P, 2], F32, kind="ExternalInput")
    bk_d = nc.dram_tensor("bk", [P, 2], F32, kind="ExternalInput")
    bv_d = nc.dram_tensor("bv", [1, H_LOC * DIM_HEAD], F32, kind="ExternalInput")
    out_d = nc.dram_tensor("out", [N, DIM], BF16, kind="ExternalOutput")

    with tile.TileContext(nc) as tc:
        ctx = ExitStack()
        with ctx:
            const = ctx.enter_context(tc.tile_pool(name="const", bufs=1))
            persist = ctx.enter_context(tc.tile_pool(name="persist", bufs=1))
            xpool = ctx.enter_context(tc.tile_pool(name="xpool", bufs=5))
            xnpool = ctx.enter_context(tc.tile_pool(name="xnpool", bufs=4))
            stat = ctx.enter_context(tc.tile_pool(name="stat", bufs=8))
            expp = ctx.enter_context(tc.tile_pool(name="expp", bufs=3))
            rbcp = ctx.enter_context(tc.tile_pool(name="rbcp", bufs=2))
            dramp = ctx.enter_context(tc.tile_pool(name="dramp", bufs=2, space="DRAM"))
            stage = ctx.enter_context(tc.tile_pool(name="stage", bufs=3))

            # ---- constants ----
            ident = const.tile([P, P], BF16, tag="ident")
            make_identity(nc, ident)
            # keep-mask for the causal diagonal block: tri[k, q] = (k <= q)
            tri = const.tile([P, P], BF16, tag="tri")
            nc.gpsimd.memset(tri[:], 0.0)
            nc.gpsimd.affine_select(
                out=tri[:], in_=tri[:], compare_op=ALU.is_gt, fill=1.0,
                base=0, channel_multiplier=1, pattern=[[-1, P]],
            )
            eps_t = const.tile([P, 1], F32, tag="eps")
            nc.vector.memset(eps_t, LN_EPS)

            # persistent tensors
            xnT = [persist.tile([P, KB, 4 * P], BF16, tag=f"xnT{q}", name=f"xnT{q}")
                   for q in range(4)]
            QTt = [persist.tile([P, N], BF16, tag=f"qt{p_}", name=f"qt{p_}")
                   for p_ in range(2)]
            KTt = [persist.tile([P, N], BF16, tag=f"kt{p_}", name=f"kt{p_}")
                   for p_ in range(2)]
            Vt = persist.tile([P, NB, H_LOC, DIM_HEAD + 1], BF16, tag="v")
            nc.gpsimd.memset(Vt[:], 1.0)  # 65th column stays 1.0 -> denominators
            outTt = [[persist.tile([P, CH], BF16, tag=f"outT{p_}_{c_}",
                                   name=f"outT{p_}_{c_}") for c_ in range(NCH)]
                     for p_ in range(2)]

            # x prefetch + weight loads, ordered so x0 leads the DMA queue
            x_tiles = {}

            def load_x(sb):
                t = xpool.tile([P, DIM], BF16, tag="x", name=f"x{sb}")
                nc.sync.dma_start(t[:], x_d[sb * P:(sb + 1) * P, :])
                x_tiles[sb] = t

            def load_w(dram, shape3, tag):
                t = persist.tile(shape3, BF16, tag=tag, name=f"w_{tag}")
                nc.sync.dma_start(t[:], dram[:].rearrange("(kb p) m -> p kb m", p=P))
                return t

            load_x(0)
            load_x(1)
            wv_sb = load_w(wv_d, [P, KB, H_LOC * DIM_HEAD], "wv")
            bq_sb = const.tile([P, 2], F32, tag="bq")
            nc.sync.dma_start(bq_sb[:], bq_d[:])
            bk_sb = const.tile([P, 2], F32, tag="bk")
            nc.sync.dma_start(bk_sb[:], bk_d[:])
            bv_sb = const.tile([P, H_LOC, DIM_HEAD], F32, tag="bv")
            nc.sync.dma_start(
                bv_sb[:],
                bv_d[:].rearrange("o (h d) -> o h d", h=H_LOC)
                .to_broadcast((P, H_LOC, DIM_HEAD)),
            )
            load_x(2)
            load_x(3)
            wq_sb = load_w(wq_d, [P, KB, H_LOC * DIM_HEAD], "wq")
            wk_sb = load_w(wk_d, [P, KB, H_LOC * DIM_HEAD], "wk")

            # ---- phase A: LN -> transpose -> QKV -> V (interleaved) ----
            psA_cm = tc.tile_pool(name="psA", bufs=6, space="PSUM")
            psA = psA_cm.__enter__()

            def emit_qkv_st(st):
                for (wt, bias_sb, dstt) in ((wq_sb, bq_sb, QTt), (wk_sb, bk_sb, KTt)):
                    for pr in range(2):
                        ps = psA.tile([P, 512], F32, tag="ps")
                        for kb in range(KB):
                            nc.tensor.matmul(
                                ps[:],
                                wt[:, kb, pr * P:(pr + 1) * P],
                                xnT[st][:, kb, :],
                                start=(kb == 0), stop=(kb == KB - 1),
                            )
                        nc.vector.tensor_scalar_add(
                            dstt[pr][:, st * 512:(st + 1) * 512], ps[:],
                            bias_sb[:, pr:pr + 1],
                        )

            for sb in range(NB):
                if sb + 4 < NB:
                    load_x(sb + 4)
                x_t = x_tiles.pop(sb)

                stats = stat.tile([P, 2, 6], F32, tag="bnst")
                x3 = x_t[:].rearrange("p (a f) -> p a f", a=2)
                for a in range(2):
                    nc.vector.bn_stats(stats[:, a, :], x3[:, a, :])
                mv = stat.tile([P, 2], F32, tag="mv")
                nc.vector.bn_aggr(mv[:], stats[:])
                rstd = stat.tile([P, 1], F32, tag="rstd")
                nc.scalar.activation(rstd[:], mv[:, 1:2], AF.Sqrt, bias=eps_t[:])
                nc.vector.reciprocal(rstd[:], rstd[:])
                # nmrs = -mean * rstd  -> xn = x*rstd + nmrs on ScalarE
                nmrs = stat.tile([P, 1], F32, tag="nmrs")
                nc.vector.tensor_scalar(
                    nmrs[:], mv[:, 0:1], rstd[:], -1.0, ALU.mult, ALU.mult
                )
                xn_bf = xnpool.tile([P, DIM], BF16, tag="xn")
                nc.scalar.activation(
                    xn_bf[:], x_t[:], AF.Identity, bias=nmrs[:], scale=rstd[:]
                )

                # transpose this seq block: 8 dim-blocks via PE, 2 psum tiles
                for half in range(2):
                    ps = psA.tile([P, 512], F32, tag="ps")
                    for j in range(4):
                        kb = half * 4 + j
                        nc.tensor.matmul(
                            ps[:, j * P:(j + 1) * P],
                            xn_bf[:, kb * P:(kb + 1) * P],
                            ident[:],
                            start=True, stop=True,
                        )
                    dst = xnT[sb // 4][:, half * 4:(half + 1) * 4,
                                       (sb % 4) * P:(sb % 4 + 1) * P]
                    src = ps[:].rearrange("p (a f) -> p a f", a=4)
                    if half == 0:
                        nc.scalar.copy(dst, src)
                    else:
                        nc.vector.tensor_copy(dst, src)

                # V for this seq block
                ps = psA.tile([P, 512], F32, tag="ps")
                psv = ps[:, :H_LOC * DIM_HEAD]
                for kb in range(KB):
                    nc.tensor.matmul(
                        psv,
                        xnT[sb // 4][:, kb, (sb % 4) * P:(sb % 4 + 1) * P],
                        wv_sb[:, kb, :],
                        start=(kb == 0), stop=(kb == KB - 1),
                    )
                nc.vector.tensor_tensor(
                    Vt[:, sb, :, :DIM_HEAD],
                    psv.rearrange("p (h d) -> p h d", h=H_LOC),
                    bv_sb[:],
                    ALU.add,
                )

                if sb % 4 == 3:
                    emit_qkv_st(sb // 4)

            wo_sb = load_w(wo_d, [P, 2, DIM], "wo")
            psA_cm.__exit__(None, None, None)

            # ---- phase B: attention per (q-chunk, head pair), software-
            # pipelined; phase C (out-proj) for chunk c-1 emitted inside
            # chunk c to keep the PE dense ----
            ctx2 = ExitStack()
            with ctx2:
                psS = ctx2.enter_context(tc.tile_pool(name="psS", bufs=2, space="PSUM"))
                psO = ctx2.enter_context(tc.tile_pool(name="psO", bufs=1, space="PSUM"))
                psP = ctx2.enter_context(tc.tile_pool(name="psP", bufs=2, space="PSUM"))

                def emit_attn_chunk(c, pr):
                    qs = c * CH
                    nkb = 4 * c + 4
                    ps_o = psO.tile([DIM_HEAD + 1, 2, CH], F32, tag="po",
                                    name=f"po_{c}_{pr}")

                    def emit_pv(kb, coff, ex):
                        for hh in range(2):
                            nc.tensor.matmul(
                                ps_o[:, hh, coff:],
                                Vt[:, kb, 2 * pr + hh, :],
                                ex[:, hh, coff:],
                                start=(kb == 0), stop=(kb == nkb - 1),
                            )

                    pend = None
                    for kb in range(nkb):
                        qlo = kb * P
                        coff = max(0, qlo - qs)
                        s_ps = psS.tile([P, 2, CH], F32, tag="sps",
                                        name=f"sps_{c}_{pr}_{kb}")
                        for hh in range(2):
                            po = hh * DIM_HEAD
                            nc.tensor.matmul(
                                s_ps[:, hh, coff:],
                                KTt[pr][po:po + DIM_HEAD, qlo:qlo + P],
                                QTt[pr][po:po + DIM_HEAD, qs + coff:qs + CH],
                                start=True, stop=True,
                                tile_position=(po, 0),
                            )
                        ex = expp.tile([P, 2, CH], BF16, tag="ex",
                                       name=f"ex_{c}_{pr}_{kb}")
                        nc.scalar.activation(ex[:, :, coff:], s_ps[:, :, coff:],
                                             AF.Exp)
                        if qlo >= qs:  # diagonal block: causal staircase mask
                            for hh in range(2):
                                nc.gpsimd.tensor_tensor(
                                    ex[:, hh, coff:coff + P],
                                    ex[:, hh, coff:coff + P],
                                    tri[:], ALU.mult,
                                )
                        if pend is not None:
                            emit_pv(*pend)
                        pend = (kb, coff, ex)
                    emit_pv(*pend)

                    # evacuate unnormalized output + denominator row, then
                    # normalize via DRAM-shuffled reciprocal broadcast
                    dr = stat.tile([1, 2, CH], F32, tag="dr", name=f"dr{c}_{pr}")
                    for hh in range(2):
                        cp = nc.scalar.copy if hh == 0 else nc.vector.tensor_copy
                        cp(outTt[pr][c][hh * DIM_HEAD:(hh + 1) * DIM_HEAD, :],
                           ps_o[:DIM_HEAD, hh, :])
                        nc.vector.tensor_copy(
                            dr[:, hh, :], ps_o[DIM_HEAD:DIM_HEAD + 1, hh, :]
                        )
                    da = dramp.tile([1, 2 * CH], F32, tag="da", name=f"da{c}_{pr}")
                    nc.sync.dma_start(da[:], dr[:].rearrange("p a f -> p (a f)"))
                    denc = stat.tile([P, 2 * CH // P], F32, tag="denc",
                                     name=f"denc{c}_{pr}")
                    nc.sync.dma_start(
                        denc[:], da[0, :].rearrange("(p o) -> p o", o=2 * CH // P)
                    )
                    nc.vector.reciprocal(denc[:], denc[:])
                    dencb = stat.tile([P, 2 * CH // P], BF16, tag="dencb",
                                      name=f"dencb{c}_{pr}")
                    nc.vector.tensor_copy(dencb[:], denc[:])
                    db = dramp.tile([1, 2 * CH], BF16, tag="db", name=f"db{c}_{pr}")
                    nc.sync.dma_start(
                        db[0, :].rearrange("(p o) -> p o", o=2 * CH // P), dencb[:]
                    )
                    recip_bc = rbcp.tile([P, CH], BF16, tag="rbc",
                                         name=f"rbc{c}_{pr}")
                    for hh in range(2):
                        nc.sync.dma_start(
                            recip_bc[hh * DIM_HEAD:(hh + 1) * DIM_HEAD, :],
                            db[:, hh * CH:(hh + 1) * CH]
                            .to_broadcast((DIM_HEAD, CH)),
                        )
                    nc.vector.tensor_tensor(
                        outTt[pr][c][:], outTt[pr][c][:], recip_bc[:], ALU.mult
                    )

                def emit_outproj_chunk(c):
                    for qb in range(4 * c, 4 * c + 4):
                        off = (qb - 4 * c) * P
                        for nt in range(2):
                            ps = psP.tile([P, 512], F32, tag="pp",
                                          name=f"pp{qb}_{nt}")
                            for pb in range(2):
                                nc.tensor.matmul(
                                    ps[:],
                                    outTt[pb][c][:, off:off + P],
                                    wo_sb[:, pb, nt * 512:(nt + 1) * 512],
                                    start=(pb == 0), stop=(pb == 1),
                                )
                            so = stage.tile([P, 512], BF16, tag="so",
                                            name=f"so{qb}_{nt}")
                            if (qb + nt) % 2 == 0:
                                nc.scalar.copy(so[:], ps[:])
                            else:
                                nc.vector.tensor_copy(so[:], ps[:])
                            nc.sync.dma_start(
                                out_d[qb * P:(qb + 1) * P,
                                      nt * 512:(nt + 1) * 512],
                                so[:],
                            )

                for c in range(NCH):
                    emit_attn_chunk(c, 0)
                    if c > 0:
                        emit_outproj_chunk(c - 1)
                    emit_attn_chunk(c, 1)
                emit_outproj_chunk(NCH - 1)

    nc.compile()
    return nc


def make_in_maps(x, ln_w, ln_b, w_qkv, w_out):
    x = np.asarray(x, np.float32)
    ln_w = np.asarray(ln_w, np.float32)
    ln_b = np.asarray(ln_b, np.float32)
    w_qkv = np.asarray(w_qkv, np.float32)
    w_out = np.asarray(w_out, np.float32)

    in_maps = []
    for c in range(N_CORES):
        b, g = c // 4, c % 4
        cols = np.arange(4 * g * DIM_HEAD, (4 * g + H_LOC) * DIM_HEAD)
        wq_s = w_qkv[:, cols]
        wk_s = w_qkv[:, INNER + cols]
        wv_s = w_qkv[:, 2 * INNER + cols]
        wq = np.ascontiguousarray(ln_w[:, None] * wq_s * SCALE)
        wk = np.ascontiguousarray(ln_w[:, None] * wk_s)
        wv = np.ascontiguousarray(ln_w[:, None] * wv_s)
        bq = (ln_b @ wq_s) * SCALE
        bk = ln_b @ wk_s
        bv = ln_b @ wv_s
        in_maps.append({
            "x": np.ascontiguousarray(x[b]).astype(BFNP),
            "wq": wq.astype(BFNP), "wk": wk.astype(BFNP), "wv": wv.astype(BFNP),
            "wo": np.ascontiguousarray(w_out[cols, :]).astype(BFNP),
            "bq": np.ascontiguousarray(bq.reshape(2, P).T),
            "bk": np.ascontiguousarray(bk.reshape(2, P).T),
            "bv": bv.reshape(1, H_LOC * DIM_HEAD),
        })
    return in_maps


_NC_CACHE = []


def kernel(x, ln_w, ln_b, w_qkv, w_out):
    in_maps = make_in_maps(x, ln_w, ln_b, w_qkv, w_out)
    if not _NC_CACHE:
        _NC_CACHE.append(build_nc())
    nc = _NC_CACHE[0]
    res = run_bass_kernel_spmd(nc, in_maps, list(range(N_CORES))).results
    out = np.zeros((B, N, DIM), np.float32)
    for c in range(N_CORES):
        out[c // 4] += np.asarray(res[c]["out"], np.float32)
    return out
